# revision 52
# baseline (speedup 1.0000x reference)
"""ACSL loss kernel for 8 TRN2 NeuronCores.

Strategy (data-parallel over N):
  Each core gets 2048 of the 16384 proposal rows. The reference loss

      L = sum_ij wm[i,j] * (softplus(x[i,j]) - x[i,j]*onehot[i,j]) / N

  (in permuted-column space) is decomposed so the device only does the
  O(N*C) work:

   - The column permutation is folded into the C-length class vectors on
     the host (roll by -1); one_hot(lab) in permuted space equals
     one_hot(labels) in original space.
   - wm at the own-label column is always 1, so the -x*onehot term is
     -sum_i x[i, labels[i]]  -> host gather, O(N).
   - fg rows: sum_j max(hs, onehot)*sp = sum_j hs*sp + (1-hs_lab)*sp_lab.
     The second piece is an O(N) host correction. The first piece uses
         hs*sp = relu(sp - t) + t*[x >= thr]      (t = softplus(thr))
     so two 4x-rate VectorE tensor_scalar ops with fused row-sum
     accumulation produce per-row partial sums; one TensorE matmul
     against the per-row fg flags reduces them (host takes the diagonal).
   - bg rows: the weight row is one of 4 vectors w(sr,sc) (host-computed,
     including the min(.,1) clip and the forced background column), so
         sum_{bg} sum_j w_k[j]*sp[i,j] = sum_k dot(w_k, colsum_k)
     where colsum_k = G_k^T @ sp is a 4-column TensorE matmul accumulated
     in PSUM across all row tiles. Host does the final dot.

  softplus itself is Ln(1*Exp(x) + 1) — two ScalarE passes sharing the
  natural_log_exp_and_others ACT table set (no softplus table exists in
  this toolchain; x ~ N(0,1) is bounded so Exp cannot overflow).

  Device pipeline per tile: DMA x (bf16) -> ScalarE Exp -> ScalarE Ln ->
  VectorE tensor_scalar x2 (accums) -> TensorE matmuls accumulating in
  PSUM. Inputs are cast to bf16 on the host (loss error ~4e-4, verified
  against the f64 reference decomposition).
"""

import sys

for _p in ("/opt/trn_rl_repo",):
    if _p not in sys.path:
        sys.path.insert(0, _p)

import numpy as np
from ml_dtypes import bfloat16

import concourse.bass as bass
import concourse.mybir as mybir
import concourse.tile as tile
from concourse.bass_utils import run_bass_kernel_spmd

N = 16384
C = 1204
NCORES = 8
ROWS_PER_CORE = N // NCORES          # 2048
P = 128                              # SBUF partitions
# tapered supertile sizes (in 128-row halves): small first so the ACT
# engine starts early (the first DMA gates it), growing as the DMA
# stream builds headroom, small last so the VE/PE tail after the final
# Ln is short.
SIZES = [1, 2, 3, 4, 3, 2, 1]
NT = len(SIZES)
NH = sum(SIZES)                      # 128-row halves per core (16)
OFFS = [sum(SIZES[:i]) for i in range(NT)]   # half-index offset per tile
THR = float(np.log(0.7 / 0.3))       # sigmoid(x) >= 0.7  <=>  x >= THR
E_THR = 0.7 / 0.3                    # x >= THR  <=>  exp(x) >= 7/3
T_SP = float(np.log(1.0 + 0.7 / 0.3))  # softplus(THR)
XBUFS, EBUFS, SPBUFS = 3, 3, 3       # tile pool double/triple buffering
SMALL_DMA_ENGINE = "gpsimd"          # queue for the small flag DMAs
EARLY_START = False                  # pre-TC early start measured neutral/worse
# column chunks for the bg matmul (PSUM bank = 512 fp32 per matmul)
CHUNKS = [(0, 512), (512, 512), (1024, C - 1024)]

_compiled = {}


def _split_waits(nc, max_waits=1):
    """Walrus codegen rejects instructions carrying more than one sem-wait
    ("Too many sync wait commands"); hoist extras onto single-wait NoOps on
    the same engine immediately before the instruction."""
    for fn in nc.m.functions:
        for blk in fn.blocks:
            out = []
            for inst in blk.instructions:
                si = inst.sync_info
                waits = list(si.on_wait) if si and si.on_wait else []
                if len(waits) > max_waits:
                    head, tail = waits[:-max_waits], waits[-max_waits:]
                    for j, w in enumerate(head):
                        out.append(mybir.InstNoOp(
                            name=f"{inst.name}-sw{j}",
                            engine=inst.engine,
                            ins=[], outs=[],
                            sync_info=mybir.SyncInfo(on_wait=[w],
                                                     on_update=[]),
                        ))
                    inst.sync_info = mybir.SyncInfo(
                        on_wait=tail, on_update=list(si.on_update or []))
                out.append(inst)
            blk.instructions = out


class _FastTailTC(tile.TileContext):
    """TileContext with a cheaper kernel tail: the stock _drain_and_barrier
    runs drain -> barrier -> gpsimd.dma_reset + sem_clear -> barrier, where
    the dma_reset drain and the second barrier cost ~5-6us. The leading
    drain already waits for every semaphore (so all DMAs have completed and
    incremented), making the DMA-state reset redundant; the second barrier
    only guards engine halt ordering, which NRT's execution-complete
    handling already provides."""

    def _drain_and_barrier(self, tick_clock, wait_clock):
        from concourse.bass import compact_to_ranges
        from concourse.vector_clock import ScopedClock

        drain_inst = self.nc.sync.drain()
        wait_clock.add_sem_waits(
            drain_inst.ins, ScopedClock({None: tick_clock.global_clock}))
        self.nc.all_engine_barrier()
        popped = self.nc._tile_sem_poison_stack.pop()
        assert popped is self._sem_poison
        sems = list(self.sems.allocated().values())
        sem_nums = [s.num if hasattr(s, "num") else int(s) for s in sems]
        sem_nums += getattr(self.nc, "_extra_clear_sems", [])
        for r in compact_to_ranges(sem_nums):
            self.nc.gpsimd.sem_clear(r)
        self.nc._state.prepend_free_semaphores(sem_nums)
        for poison_set in self.nc._tile_sem_poison_stack:
            poison_set.update(sem_nums)


def _build_graph():
    from contextlib import ExitStack

    nc = bass.Bass()
    x_d = nc.dram_tensor("x", [ROWS_PER_CORE, C], mybir.dt.bfloat16,
                         kind="ExternalInput")
    gb_d = nc.dram_tensor("gbg", [P, NH * 4], mybir.dt.bfloat16,
                          kind="ExternalInput")
    gf_d = nc.dram_tensor("gfg", [P, NH], mybir.dt.float32,
                          kind="ExternalInput")
    out_d = nc.dram_tensor("out", [4 + NH, C], mybir.dt.float32,
                           kind="ExternalOutput")

    F = mybir.ActivationFunctionType

    ctx = ExitStack()
    if EARLY_START:
        # Pre-TileContext warmup, executed right after program load and
        # BEFORE the Tile preamble barriers (~7us in):
        #  - dummy Exp forces walrus to place the ACT table load here
        #  - the first x tile's DMA is triggered immediately (its
        #    completion is signalled on a raw semaphore the first real
        #    Exp waits on)
        r0 = SIZES[0]
        xt0 = ctx.enter_context(
            nc.sbuf_tensor("xt0", [P, r0 * C], mybir.dt.bfloat16))
        dma_sem = ctx.enter_context(nc.semaphore("early_dma"))
        nc.sync.dma_start(
            xt0[:],
            x_d[0:r0 * P, :].rearrange("(p r) c -> p (r c)", p=P, r=r0),
        ).then_inc(dma_sem, 16)
        # must be zeroed at kernel tail or a re-execution's wait_ge(16)
        # would pass before the fresh DMA lands
        nc._extra_clear_sems = [dma_sem.num]

    with _FastTailTC(nc) as tc:
        with (
            tc.tile_pool(name="xin", bufs=XBUFS) as xpool,
            tc.tile_pool(name="e", bufs=EBUFS) as epool,
            tc.tile_pool(name="sp", bufs=SPBUFS) as sppool,
            tc.tile_pool(name="scr", bufs=2) as scrpool,
            tc.tile_pool(name="small", bufs=1) as smpool,
            tc.tile_pool(name="g", bufs=2) as gpool,
            tc.tile_pool(name="ps", bufs=1, space="PSUM") as pspool,
        ):
            psum_bg = pspool.tile([4, C], mybir.dt.float32, tag="psbg")
            psum_fg = pspool.tile([NH, NH], mybir.dt.float32, tag="psfg")
            # per-half row-sums of hs*sp (STT accum_out = sum(out))
            accstrip = smpool.tile([P, NH], mybir.dt.float32, tag="acc")
            gf = smpool.tile([P, NH], mybir.dt.float32, tag="gf")
            # all per-half flag vectors arrive in two strip DMAs on
            # GpSimd's SWDGE queue (the Sync HWDGE queue only streams x)
            gb = smpool.tile([P, NH * 4], mybir.dt.bfloat16, tag="gb")
            small_eng = getattr(nc, SMALL_DMA_ENGINE)
            small_eng.dma_start(gb[:], gb_d[:])
            small_eng.dma_start(gf[:], gf_d[:])

            from concourse.bass import _add_dep_helper
            prev_ln = None
            for s, rs in enumerate(SIZES):
                o = OFFS[s]
                rows = slice(o * P, (o + rs) * P)
                if EARLY_START and s == 0:
                    # the wait on the early-DMA sem is injected after
                    # scheduling (Tile's sim can't see the external inc)
                    xt = xt0
                else:
                    xt = xpool.tile([P, rs * C], mybir.dt.bfloat16, tag="x")
                    nc.sync.dma_start(
                        xt[:],
                        x_d[rows, :].rearrange("(p r) c -> p (r c)",
                                               p=P, r=rs))

                et = epool.tile([P, rs * C], mybir.dt.bfloat16, tag="e")
                exp_i = nc.scalar.activation(et[:], xt[:], F.Exp)
                if EARLY_START and s == 0:
                    _compiled["exp0_name"] = exp_i.ins.name
                if prev_ln is not None:
                    # keep the ACT engine alternating exp/ln per supertile
                    # so VectorE gets sp tiles as early as possible
                    _add_dep_helper(exp_i.ins, prev_ln.ins, sync=False,
                                    reason="ACT order: ln_s before exp_s+1")
                spt = sppool.tile([P, rs * C], mybir.dt.bfloat16, tag="sp")
                prev_ln = nc.scalar.activation(spt[:], et[:], F.Ln, bias=1.0)

                for r in range(rs):
                    h = o + r
                    sl = slice(r * C, (r + 1) * C)
                    # accstrip[:,h] = sum_j (exp(x) >= 7/3) * sp
                    sq = scrpool.tile([P, C], mybir.dt.bfloat16, tag="sq")
                    nc.vector.scalar_tensor_tensor(
                        out=sq[:], in0=et[:, sl], scalar=E_THR,
                        in1=spt[:, sl],
                        op0=mybir.AluOpType.is_ge,
                        op1=mybir.AluOpType.mult,
                        accum_out=accstrip[:, h:h + 1],
                    )
                    for c0, cw in CHUNKS:
                        nc.tensor.matmul(
                            psum_bg[0:4, c0:c0 + cw],
                            lhsT=gb[:, h * 4:(h + 1) * 4],
                            rhs=spt[:, r * C + c0: r * C + c0 + cw],
                            start=(h == 0),
                            stop=(h == NH - 1),
                        )

            # fg reduction: psum_fg = gf^T @ accstrip, host takes the diag
            nc.tensor.matmul(psum_fg[:, :], lhsT=gf[:], rhs=accstrip[:],
                             start=True, stop=True)

            # stage both results in one tile -> single output DMA; the
            # PSUM->SBUF copies are split across VectorE and ScalarE
            # copies pipelined per column chunk: each chunk's PSUM
            # accumulation completes at its own last matmul, so its copy
            # starts while later chunks are still accumulating
            out_bg = smpool.tile([4, C], mybir.dt.float32, tag="obg")
            out_fg = smpool.tile([NH, NH], mybir.dt.float32, tag="ofg")
            nc.vector.tensor_copy(out_bg[:, 0:512], psum_bg[:, 0:512])
            nc.scalar.copy(out_bg[:, 512:1024], psum_bg[:, 512:1024])
            nc.vector.tensor_copy(out_bg[:, 1024:C], psum_bg[:, 1024:C])
            nc.scalar.copy(out_fg[:], psum_fg[:])
            nc.sync.dma_start(out_d[0:4, :], out_bg[:])
            nc.sync.dma_start(out_d[4:4 + NH, 0:NH], out_fg[:])
    ctx.close()
    if EARLY_START:
        # inject the ACT-engine wait on the early-DMA semaphore right
        # before the first Exp (post-scheduling, invisible to Tile's sim)
        w = mybir.SyncWait(sync_type="semaphore", id=dma_sem.num,
                           ant_name="early_dma", wait_mode="sem-ge-imm",
                           wait_value=16)
        target = _compiled.pop("exp0_name")
        for fn in nc.m.functions:
            for blk in fn.blocks:
                if any(i.name == target for i in blk.instructions):
                    out = []
                    for inst in blk.instructions:
                        if inst.name == target:
                            out.append(mybir.InstNoOp(
                                name=f"{target}-earlywait",
                                engine=inst.engine, ins=[], outs=[],
                                sync_info=mybir.SyncInfo(on_wait=[w],
                                                         on_update=[])))
                        out.append(inst)
                    blk.instructions = out
    _split_waits(nc)
    return nc


def _get_graph():
    if "nc" not in _compiled:
        _compiled["nc"] = _build_graph()
    return _compiled["nc"]


def _prep(cls_logits, labels, rare_sel, common_sel, rare_vec, common_vec,
          freq_vec):
    """Host-side preprocessing. Returns (in_maps, W, host_const)."""
    x = np.asarray(cls_logits, np.float32)
    labels = np.asarray(labels).astype(np.int64)
    rare_sel = np.asarray(rare_sel).astype(bool)
    common_sel = np.asarray(common_sel).astype(bool)

    # class vectors rolled from permuted space to original column space
    rare_o = np.roll(np.asarray(rare_vec).astype(np.float64), -1)
    common_o = np.roll(np.asarray(common_vec).astype(np.float64), -1)
    freq_o = np.roll(np.asarray(freq_vec).astype(np.float64), -1)

    def wvec(sr, sc):
        w = np.minimum(freq_o + sr * rare_o + sc * common_o, 1.0)
        w[C - 1] = 1.0  # permuted col 0 (background) -> original col C-1
        return w

    W = np.stack([wvec(0, 0), wvec(1, 0), wvec(0, 1), wvec(1, 1)])  # k=sr+2sc

    is_bg = labels == C - 1
    fg = ~is_bg
    k = rare_sel.astype(np.int64) + 2 * common_sel.astype(np.int64)

    # host O(N) corrections (f64)
    g = x[np.arange(N), labels].astype(np.float64)
    own_term = -np.sum(g)
    g_hs = (g >= THR)
    fg_corr = float(np.sum((np.logaddexp(0.0, g) * (1.0 - g_hs))[fg]))
    host_const = own_term + fg_corr

    # per-row device flags
    G = np.zeros((N, 4), np.float32)
    G[np.arange(N)[is_bg], k[is_bg]] = 1.0
    fgf = fg.astype(np.float32)

    xb = x.astype(bfloat16)
    in_maps = []
    for c in range(NCORES):
        rows = slice(c * ROWS_PER_CORE, (c + 1) * ROWS_PER_CORE)
        # supertile s covers rows [OFFS[s]*P, (OFFS[s]+rs)*P) laid out as
        # partition p <-> row OFFS[s]*P + p*rs + r, half index h = OFFS[s]+r
        fgc = fgf[rows]
        Gc = G[rows]
        gfg = np.zeros((P, NH), np.float32)
        gbg = np.zeros((P, NH * 4), np.float32)
        for s, rs in enumerate(SIZES):
            o = OFFS[s]
            gfg[:, o:o + rs] = fgc[o * P:(o + rs) * P].reshape(P, rs)
            gbg[:, o * 4:(o + rs) * 4] = (
                Gc[o * P:(o + rs) * P].reshape(P, rs * 4))
        in_maps.append({
            "x": np.ascontiguousarray(xb[rows]),
            "gbg": gbg.astype(bfloat16),
            "gfg": gfg,
        })
    return in_maps, W, host_const


def _reduce(results, W, host_const):
    total = host_const
    for res in results:
        out = np.asarray(res["out"], np.float64)
        total += float(np.sum(W * out[0:4]))
        total += float(np.trace(out[4:4 + NH, 0:NH]))
    return np.float32(total / N)


def kernel(cls_logits, labels, rare_sel, common_sel, rare_vec, common_vec,
           freq_vec, _run_kwargs=None):
    in_maps, W, host_const = _prep(cls_logits, labels, rare_sel, common_sel,
                                   rare_vec, common_vec, freq_vec)
    nc = _get_graph()
    kw = dict(_run_kwargs or {})
    res = run_bass_kernel_spmd(nc, in_maps, core_ids=list(range(NCORES)), **kw)
    out = _reduce(res.results, W, host_const)
    if kw:
        _compiled["last_results"] = res
    return out
